# revision 1
# baseline (speedup 1.0000x reference)
"""Trainium2 Bass kernel for 2-layer GCN forward (Reddit-like), 8-way node-sharded.

Strategy (matches the sharding hint):
- Nodes partitioned contiguously across 8 cores (12500 each). Edges (with
  self-loops) are routed to the core owning their destination node; within a
  core they are grouped by 128-node destination tile and by source "bank"
  (h tables are gathered with int16 indices, so the 100000-row feature table
  is split into 4 banks of 25000 rows).
- Per core: h1 = x_shard @ W1 on PE, AllGather the 12.8k x 64 shard ->
  full h1 table in DRAM; per edge-chunk of 128 edges, dma_gather the source
  rows, scale by the symmetric GCN norm, and segment-sum via a selection
  matrix matmul accumulated in PSUM (S[e, n] = (dst_rel[e] == n)).
  relu(+b1), @W2, AllGather again, same aggregation, +b2, log_softmax.
- Weight matrices are replicated; all index preprocessing is host-side numpy.
"""
import math
import numpy as np
from contextlib import ExitStack

import concourse.bass as bass
import concourse.bacc as bacc
import concourse.tile as tile
from concourse import mybir
from concourse.bass_utils import run_bass_kernel_spmd

# problem sizes (hardcoded per the contract)
N = 100000
E = 1250000
F_IN = 602
F_PAD = 640          # 5 x 128
HID = 64
C = 41
N_CORES = 8
NT = N // N_CORES    # 12500 nodes per core
P = 128
N_TILES = (NT + P - 1) // P          # 98 (last tile 84 nodes)
# int16-addressable h-table banks (rebalanced so per-(tile,bank) edge counts
# sit just under a multiple of 128; self-loops are handled as a diagonal term)
BANK_BOUNDS = [0, 29500, 59000, 88500, 100000]
N_BANKS = 4
GROUP = 6                            # node tiles per gather call group

PROFILE = False      # set True from test harness to request an NTFF trace
SIM_MODE = False     # replace collectives with local copies (TimelineSim only)
_LAST_RESULTS = {}   # debug: profile info stash


def _preprocess(x, src, dst, W1, b1, W2, b2):
    """Host-side index preprocessing and sharding. Returns in_maps + plan."""
    src = np.asarray(src).astype(np.int64).ravel()
    dst = np.asarray(dst).astype(np.int64).ravel()
    x = np.asarray(x, dtype=np.float32)
    W1 = np.asarray(W1, dtype=np.float32)
    b1 = np.asarray(b1, dtype=np.float32)
    W2 = np.asarray(W2, dtype=np.float32)
    b2 = np.asarray(b2, dtype=np.float32)

    # degrees include self-loops (reference semantics); the self-loop term
    # itself is applied on-device as a diagonal matmul, not via gathers.
    deg = (np.bincount(dst, minlength=N) + 1.0).astype(np.float32)
    dinv = (1.0 / np.sqrt(deg)).astype(np.float32)
    s = src
    d = dst
    norm = (dinv[s] * dinv[d]).astype(np.float32)
    bounds = np.asarray(BANK_BOUNDS[1:-1], dtype=np.int64)

    core = d // NT
    trel = (d % NT) // P
    bank = np.searchsorted(bounds, s, side="right")
    key = ((core * N_TILES + trel) * N_BANKS + bank).astype(np.int64)
    order = np.argsort(key, kind="stable")
    s_o = s[order]
    d_o = d[order]
    n_o = norm[order]
    key_o = key[order]

    n_keys = N_CORES * N_TILES * N_BANKS
    counts = np.bincount(key_o, minlength=n_keys).reshape(N_CORES, N_TILES, N_BANKS)
    # uniform chunk counts across cores (single SPMD program)
    cols = np.ceil(counts.max(axis=0) / P).astype(np.int64)      # [N_TILES, N_BANKS]
    tb_start = np.concatenate([[0], np.cumsum(cols.ravel())])[:-1].reshape(
        N_TILES, N_BANKS
    )
    NCH = int(cols.sum())                                         # chunks per core

    # group boundaries for gather calls: groups of GROUP node tiles
    groups = [
        list(range(g, min(g + GROUP, N_TILES))) for g in range(0, N_TILES, GROUP)
    ]

    # re-order chunks canonically: for g: for b: for t in g: cols[t][b] chunks
    # chunk_index[t][b] = start in canonical order
    ch_index = np.zeros((N_TILES, N_BANKS), dtype=np.int64)
    call_plan = []  # per group: list of (b, ch0, n_chunks) with ch0 canonical
    group_start = []
    cursor = 0
    for g in groups:
        group_start.append(cursor)
        for b in range(N_BANKS):
            cw = int(cols[np.array(g), b].sum())
            if cw:
                call_plan.append((len(group_start) - 1, b, cursor, cw))
            for t in g:
                ch_index[t, b] = cursor
                cursor += int(cols[t, b])
    assert cursor == NCH

    # per-core edge placement
    group_ofs = np.concatenate([[0], np.cumsum(counts.reshape(N_CORES, -1).ravel())])
    in_maps = []
    for c in range(N_CORES):
        lo = np.searchsorted(key_o, c * N_TILES * N_BANKS)
        hi = np.searchsorted(key_o, (c + 1) * N_TILES * N_BANKS)
        ks = key_o[lo:hi] - c * N_TILES * N_BANKS       # (t * N_BANKS + b)
        # position within the (t,b) run for each edge
        run_starts = np.concatenate(
            [[0], np.cumsum(np.bincount(ks, minlength=N_TILES * N_BANKS))]
        )[:-1]
        i_local = np.arange(hi - lo) - run_starts[ks]
        chv = ch_index.ravel()[ks] + i_local // P
        pv = (i_local % P).astype(np.int64)

        erel = np.zeros((P, NCH), dtype=np.float32)
        enrm = np.zeros((P, NCH), dtype=np.float32)
        idx16 = np.zeros((16, NCH * 8), dtype=np.int16)
        drel = (d_o[lo:hi] - (c * NT + (ks // N_BANKS) * P)).astype(np.float32)
        erel[pv, chv] = drel
        enrm[pv, chv] = n_o[lo:hi]
        bank_lo = np.asarray(BANK_BOUNDS, dtype=np.int64)[(ks % N_BANKS)]
        bloc = (s_o[lo:hi] - bank_lo).astype(np.int16)
        idx16[pv % 16, chv * 8 + pv // 16] = bloc
        idx128 = np.tile(idx16, (8, 1))

        xT = np.zeros((F_PAD, NT), dtype=np.float32)
        xT[:F_IN, :] = x[c * NT : (c + 1) * NT, :].T
        W1p = np.zeros((F_PAD, HID), dtype=np.float32)
        W1p[:F_IN, :] = W1
        W2p = np.zeros((HID, HID), dtype=np.float32)
        W2p[:, :C] = W2
        iota = np.tile(np.arange(P, dtype=np.float32), (P, 1))
        dv2 = np.zeros((P, N_TILES), dtype=np.float32)
        own = dinv[c * NT : (c + 1) * NT] ** 2
        dv2.T.ravel()[: NT] = own  # [t, p] row-major fill -> transpose view
        dv2 = np.zeros((P, N_TILES), dtype=np.float32)
        for t in range(N_TILES):
            tsz = min(P, NT - t * P)
            dv2[:tsz, t] = own[t * P : t * P + tsz]
        in_maps.append(
            dict(
                xT=np.ascontiguousarray(xT),
                dinv2=dv2,
                ident=np.eye(P, dtype=np.float32),
                W1p=W1p,
                b1=b1.reshape(HID, 1).astype(np.float32),
                W2p=W2p,
                b2bc=np.tile(b2.reshape(1, C), (P, 1)).astype(np.float32),
                iota=iota,
                idx16=idx128,
                erel=erel,
                enrm=enrm,
            )
        )
    plan = dict(NCH=NCH, cols=cols, ch_index=ch_index, groups=groups,
                group_start=group_start, call_plan=call_plan)
    return in_maps, plan


def _build(plan):
    NCH = plan["NCH"]
    cols = plan["cols"]
    ch_index = plan["ch_index"]
    groups = plan["groups"]
    group_start = plan["group_start"]
    call_plan = plan["call_plan"]
    f32 = mybir.dt.float32

    nc = bacc.Bacc("TRN2", target_bir_lowering=False, num_devices=N_CORES)
    xT = nc.declare_dram_parameter("xT", [F_PAD, NT], f32, isOutput=False)
    W1p = nc.declare_dram_parameter("W1p", [F_PAD, HID], f32, isOutput=False)
    b1p = nc.declare_dram_parameter("b1", [HID, 1], f32, isOutput=False)
    W2p = nc.declare_dram_parameter("W2p", [HID, HID], f32, isOutput=False)
    b2bc = nc.declare_dram_parameter("b2bc", [P, C], f32, isOutput=False)
    iota_in = nc.declare_dram_parameter("iota", [P, P], f32, isOutput=False)
    ident_in = nc.declare_dram_parameter("ident", [P, P], f32, isOutput=False)
    dinv2_in = nc.declare_dram_parameter("dinv2", [P, N_TILES], f32, isOutput=False)
    idx16_in = nc.declare_dram_parameter("idx16", [P, NCH * 8], mybir.dt.int16, isOutput=False)
    erel_in = nc.declare_dram_parameter("erel", [P, NCH], f32, isOutput=False)
    enrm_in = nc.declare_dram_parameter("enrm", [P, NCH], f32, isOutput=False)
    out_p = nc.declare_dram_parameter("out", [NT, C], f32, isOutput=True)

    h1_local = nc.dram_tensor("h1_local", [NT, HID], f32)
    h1_full = nc.dram_tensor("h1_full", [N, HID], f32, addr_space="Shared")
    h2_local = nc.dram_tensor("h2_local", [NT, HID], f32)
    h2_full = nc.dram_tensor("h2_full", [N, HID], f32, addr_space="Shared")

    rg = [list(range(N_CORES))]
    max_cols_tb = int(cols.max())
    max_cols_g = max(
        int(cols[np.array(g), :].sum()) for g in groups
    )

    with tile.TileContext(nc) as tc, ExitStack() as ctx:
        consts = ctx.enter_context(tc.tile_pool(name="consts", bufs=1))
        xpool = ctx.enter_context(tc.tile_pool(name="xpool", bufs=3))
        psA = ctx.enter_context(tc.tile_pool(name="psA", bufs=2, space="PSUM"))
        h1out = ctx.enter_context(tc.tile_pool(name="h1out", bufs=3))
        big = ctx.enter_context(tc.tile_pool(name="big", bufs=1))
        gpool = ctx.enter_context(tc.tile_pool(name="gpool", bufs=2))
        spool = ctx.enter_context(tc.tile_pool(name="spool", bufs=3))
        mpool = ctx.enter_context(tc.tile_pool(name="mpool", bufs=3))
        ps1 = ctx.enter_context(tc.tile_pool(name="ps1", bufs=2, space="PSUM"))
        smalls = ctx.enter_context(tc.tile_pool(name="smalls", bufs=4))

        # ---- constants ----
        iota_t = consts.tile([P, P], f32)
        nc.sync.dma_start(out=iota_t[:], in_=iota_in[:])
        ident_t = consts.tile([P, P], f32)
        nc.sync.dma_start(out=ident_t[:], in_=ident_in[:])
        dinv2_t = consts.tile([P, N_TILES], f32)
        nc.sync.dma_start(out=dinv2_t[:], in_=dinv2_in[:])
        W1t = consts.tile([P, 5 * HID], f32)
        W1t3 = W1t[:].rearrange("p (c h) -> p c h", c=5)
        nc.sync.dma_start(
            out=W1t3, in_=W1p[:].rearrange("(c p) h -> p c h", c=5)
        )
        b1t = consts.tile([HID, 1], f32)
        nc.sync.dma_start(out=b1t[:], in_=b1p[:])
        W2t = consts.tile([HID, HID], f32)
        nc.sync.dma_start(out=W2t[:], in_=W2p[:])
        b2t = consts.tile([P, C], f32)
        nc.sync.dma_start(out=b2t[:], in_=b2bc[:])
        idxt = consts.tile([P, NCH * 8], mybir.dt.int16)
        nc.sync.dma_start(out=idxt[:], in_=idx16_in[:])
        erelt = consts.tile([P, NCH], f32)
        nc.sync.dma_start(out=erelt[:], in_=erel_in[:])
        enrmt = consts.tile([P, NCH], f32)
        nc.sync.dma_start(out=enrmt[:], in_=enrm_in[:])

        xT3 = xT[:].rearrange("(c p) n -> p c n", c=5)

        # ---- phase 1: h1_local = x @ W1 ----
        for t in range(N_TILES):
            t0 = t * P
            tsz = min(P, NT - t0)
            xt = xpool.tile([P, 5 * P], f32, tag="xt")
            xt3 = xt[:].rearrange("p (c n) -> p c n", c=5)
            nc.sync.dma_start(out=xt3[:, :, :tsz], in_=xT3[:, :, t0 : t0 + tsz])
            pa = psA.tile([P, HID], f32)
            for cb in range(5):
                nc.tensor.matmul(
                    out=pa[:tsz, :],
                    lhsT=xt3[:, cb, :tsz],
                    rhs=W1t3[:, cb, :],
                    start=(cb == 0),
                    stop=(cb == 4),
                )
            h1sb = h1out.tile([P, HID], f32, tag="h1sb")
            nc.scalar.activation(
                out=h1sb[:tsz, :], in_=pa[:tsz, :],
                func=mybir.ActivationFunctionType.Copy,
            )
            nc.sync.dma_start(out=h1_local[t0 : t0 + tsz, :], in_=h1sb[:tsz, :])

        if SIM_MODE:
            nc.sync.dma_start(out=h1_full[0:NT, :], in_=h1_local[:])
        else:
            nc.gpsimd.collective_compute(
                "AllGather", mybir.AluOpType.bypass, replica_groups=rg,
                ins=[h1_local[:]], outs=[h1_full[:]],
            )

        # persistent relu(h1_agg)^T  [HID, NT]
        h1rT = big.tile([HID, NT], f32)

        def conv(layer, h_full, h_local, gi):
            """Aggregation sweep. layer=1: out h1rT (transposed, relu+b1).
            layer=2: +b2, log_softmax, DMA to out_p."""
            for gidx, g in enumerate(groups):
                gs = group_start[gidx]
                g_cols = int(cols[np.array(g), :].sum())
                gout = gpool.tile([P, max_cols_g * HID], f32, tag="gout")
                g3 = gout[:].rearrange("p (c h) -> p c h", c=max_cols_g)
                for (gg, b, ch0, cw) in call_plan:
                    if gg != gidx:
                        continue
                    loc = ch0 - gs
                    nc.gpsimd.dma_gather(
                        out_ap=g3[:, loc : loc + cw, :],
                        in_ap=h_full[BANK_BOUNDS[b] : BANK_BOUNDS[b + 1], :],
                        idxs_ap=idxt[:, ch0 * 8 : (ch0 + cw) * 8],
                        num_idxs=cw * P,
                        num_idxs_reg=cw * P,
                        elem_size=HID,
                        single_packet=False,
                    )
                for t in g:
                    t0 = t * P
                    tsz = min(P, NT - t0)
                    n_ch_t = int(cols[t, :].sum()) + 1
                    if layer == 1:
                        pt = ps1.tile([HID, P], f32, tag="ps_l1")
                    else:
                        pt = ps1.tile([P, HID], f32, tag="ps_l2")
                    # self-loop diagonal term: D = diag(dinv^2) over this tile,
                    # own rows come from the core-local h tensor
                    hown = mpool.tile([P, HID], f32, tag="hown")
                    nc.sync.dma_start(
                        out=hown[:tsz, :], in_=h_local[t0 : t0 + tsz, :]
                    )
                    DD = spool.tile([P, P], f32, tag="DD")
                    nc.vector.tensor_scalar(
                        out=DD[:], in0=ident_t[:], scalar1=dinv2_t[:, t : t + 1],
                        scalar2=None, op0=mybir.AluOpType.mult,
                    )
                    if layer == 1:
                        nc.tensor.matmul(
                            out=pt[:], lhsT=hown[:], rhs=DD[:],
                            start=True, stop=(n_ch_t == 1),
                        )
                    else:
                        nc.tensor.matmul(
                            out=pt[:], lhsT=DD[:], rhs=hown[:],
                            start=True, stop=(n_ch_t == 1),
                        )
                    k = 1
                    for b in range(N_BANKS):
                        cw = int(cols[t, b])
                        if cw == 0:
                            continue
                        ch0 = int(ch_index[t, b])
                        loc = ch0 - gs
                        for j in range(cw):
                            ch = ch0 + j
                            # norm-scaled one-hot: S[p, n] = (iota==dst_rel[p]) * norm[p]
                            SS = spool.tile([P, P], f32, tag="SS")
                            nc.vector.tensor_scalar(
                                out=SS[:], in0=iota_t[:],
                                scalar1=erelt[:, ch : ch + 1],
                                scalar2=enrmt[:, ch : ch + 1],
                                op0=mybir.AluOpType.is_equal,
                                op1=mybir.AluOpType.mult,
                            )
                            if layer == 1:
                                nc.tensor.matmul(
                                    out=pt[:],
                                    lhsT=g3[:, loc + j, :],
                                    rhs=SS[:],
                                    start=False,
                                    stop=(k == n_ch_t - 1),
                                )
                            else:
                                nc.tensor.matmul(
                                    out=pt[:],
                                    lhsT=SS[:],
                                    rhs=g3[:, loc + j, :],
                                    start=False,
                                    stop=(k == n_ch_t - 1),
                                )
                            k += 1
                    if layer == 1:
                        nc.scalar.activation(
                            out=h1rT[:, t0 : t0 + tsz], in_=pt[:, :tsz],
                            func=mybir.ActivationFunctionType.Relu,
                            bias=b1t[:],
                        )
                    else:
                        # L = psum + b2 into the batched logits buffer
                        nc.vector.tensor_tensor(
                            out=Lb3[:tsz, t, :], in0=pt[:tsz, :C],
                            in1=b2t[:tsz, :], op=mybir.AluOpType.add,
                        )

        conv(1, h1_full, h1_local, 0)

        # ---- layer 2 linear: h2_local = relu(h1_agg) @ W2 (zero-padded cols) ----
        for t in range(N_TILES):
            t0 = t * P
            tsz = min(P, NT - t0)
            pb = psA.tile([P, HID], f32, tag="ps_l2lin")
            nc.tensor.matmul(
                out=pb[:tsz, :], lhsT=h1rT[:, t0 : t0 + tsz], rhs=W2t[:],
                start=True, stop=True,
            )
            h2sb = h1out.tile([P, HID], f32, tag="h2sb")
            nc.scalar.activation(
                out=h2sb[:tsz, :], in_=pb[:tsz, :],
                func=mybir.ActivationFunctionType.Copy,
            )
            nc.sync.dma_start(out=h2_local[t0 : t0 + tsz, :], in_=h2sb[:tsz, :])

        if SIM_MODE:
            nc.sync.dma_start(out=h2_full[0:NT, :], in_=h2_local[:])
        else:
            nc.gpsimd.collective_compute(
                "AllGather", mybir.AluOpType.bypass, replica_groups=rg,
                ins=[h2_local[:]], outs=[h2_full[:]],
            )

        Lbig = big.tile([P, N_TILES * C], f32)
        Lb3 = Lbig[:].rearrange("p (t c) -> p t c", t=N_TILES)

        conv(2, h2_full, h2_local, 1)

        # ---- batched log_softmax over all tiles ----
        negm = big.tile([P, N_TILES], f32)
        nc.vector.tensor_reduce(
            out=negm[:], in_=Lb3, axis=mybir.AxisListType.X,
            op=mybir.AluOpType.max, negate=True,
        )
        # Lc = L - max (3D broadcast of negm), in place
        Lc = Lbig
        Lc3 = Lb3
        nc.vector.tensor_tensor(
            out=Lc3, in0=Lb3, in1=negm[:].to_broadcast([P, N_TILES, C]),
            op=mybir.AluOpType.add,
        )
        Eb = big.tile([P, N_TILES * C], f32)
        nc.scalar.activation(
            out=Eb[:], in_=Lc[:], func=mybir.ActivationFunctionType.Exp,
        )
        sums = big.tile([P, N_TILES], f32)
        nc.vector.tensor_reduce(
            out=sums[:], in_=Eb[:].rearrange("p (t c) -> p t c", t=N_TILES),
            axis=mybir.AxisListType.X, op=mybir.AluOpType.add,
        )
        lns = big.tile([P, N_TILES], f32)
        nc.scalar.activation(
            out=lns[:], in_=sums[:], func=mybir.ActivationFunctionType.Ln,
        )
        # out = Lc - ln(sum)
        nc.vector.tensor_tensor(
            out=Lc3, in0=Lc3, in1=lns[:].to_broadcast([P, N_TILES, C]),
            op=mybir.AluOpType.subtract,
        )
        # two DMAs: full tiles then the 84-row tail (rows beyond NT are garbage)
        nc.sync.dma_start(
            out=out_p[0 : (N_TILES - 1) * P, :].rearrange("(t p) c -> p t c", t=N_TILES - 1),
            in_=Lc3[:, : N_TILES - 1, :],
        )
        last0 = (N_TILES - 1) * P
        nc.sync.dma_start(
            out=out_p[last0:NT, :], in_=Lc3[: NT - last0, N_TILES - 1, :],
        )

    nc.compile()
    return nc


def kernel(x, src, dst, W1, b1, W2, b2):
    in_maps, plan = _preprocess(x, src, dst, W1, b1, W2, b2)
    nc = _build(plan)
    res = run_bass_kernel_spmd(
        nc, in_maps, list(range(N_CORES)), trace=PROFILE
    )
    _LAST_RESULTS["exec_time_ns"] = getattr(res, "exec_time_ns", None)
    _LAST_RESULTS["profile_json"] = getattr(res, "profile_json", None)
    out = np.concatenate([res.results[c]["out"] for c in range(N_CORES)], axis=0)
    return out.astype(np.float32)



# revision 2
# speedup vs baseline: 5.2882x; 5.2882x over previous
"""Trainium2 Bass kernel for 2-layer GCN forward (Reddit-like), 8-way node-sharded.

Strategy (matches the sharding hint):
- Nodes partitioned contiguously across 8 cores (12500 each). Edges (with
  self-loops) are routed to the core owning their destination node; within a
  core they are grouped by 128-node destination tile and by source "bank"
  (h tables are gathered with int16 indices, so the 100000-row feature table
  is split into 4 banks of <32768 rows).
- Per core: h1 = x_shard @ W1 on PE (bf16), AllGather the 12.5k x 128 shard ->
  full h1 table in DRAM; per edge-chunk of 128 edges, dma_gather the source
  rows, and segment-sum via a norm-scaled one-hot selection matrix matmul
  accumulated in PSUM (S[e, n] = (dst_rel[e] == n) * norm[e]).
  relu(+b1), @W2, AllGather again, same aggregation, +b2, log_softmax.
- All tensors feeding the PE are bf16 (1 cycle/row vs 4 for fp32); PSUM
  accumulation stays fp32. h tables are padded to 128 cols so gather rows are
  256B (DMA gather requires elem_size and row stride to be multiples of 256B).
- Weight matrices are replicated; all index preprocessing is host-side numpy.
"""
import math
import numpy as np
import ml_dtypes
from contextlib import ExitStack

import concourse.bass as bass
import concourse.bacc as bacc
import concourse.tile as tile
from concourse import mybir
from concourse.bass_utils import run_bass_kernel_spmd

BF16 = ml_dtypes.bfloat16

# problem sizes (hardcoded per the contract)
N = 100000
E = 1250000
F_IN = 602
F_PAD = 640          # 5 x 128
HID = 64
HID2 = 128           # padded h-table width (256B bf16 rows)
C = 41
N_CORES = 8
NT = N // N_CORES    # 12500 nodes per core
P = 128
N_TILES = (NT + P - 1) // P          # 98 (last tile 84 nodes)
# int16-addressable h-table banks
BANK_BOUNDS = [0, 29500, 59000, 88500, 100000]
N_BANKS = 4
GROUP = 6                            # node tiles per gather call group

PROFILE = False      # set True from test harness to request an NTFF trace
SIM_MODE = False     # replace collectives with local copies (TimelineSim only)
_LAST_RESULTS = {}   # debug: profile info stash


def _preprocess(x, src, dst, W1, b1, W2, b2):
    """Host-side index preprocessing and sharding. Returns in_maps + plan."""
    src = np.asarray(src).astype(np.int64).ravel()
    dst = np.asarray(dst).astype(np.int64).ravel()
    x = np.asarray(x, dtype=np.float32)
    W1 = np.asarray(W1, dtype=np.float32)
    b1 = np.asarray(b1, dtype=np.float32)
    W2 = np.asarray(W2, dtype=np.float32)
    b2 = np.asarray(b2, dtype=np.float32)

    # degrees include self-loops (reference semantics); the self-loop term
    # itself is applied on-device as a diagonal matmul, not via gathers.
    deg = (np.bincount(dst, minlength=N) + 1.0).astype(np.float32)
    dinv = (1.0 / np.sqrt(deg)).astype(np.float32)
    s = src
    d = dst
    norm = (dinv[s] * dinv[d]).astype(np.float32)
    bounds = np.asarray(BANK_BOUNDS[1:-1], dtype=np.int64)

    core = d // NT
    trel = (d % NT) // P
    bank = np.searchsorted(bounds, s, side="right")
    key = ((core * N_TILES + trel) * N_BANKS + bank).astype(np.int64)
    order = np.argsort(key, kind="stable")
    s_o = s[order]
    d_o = d[order]
    n_o = norm[order]
    key_o = key[order]

    n_keys = N_CORES * N_TILES * N_BANKS
    counts = np.bincount(key_o, minlength=n_keys).reshape(N_CORES, N_TILES, N_BANKS)
    # uniform chunk counts across cores (single SPMD program)
    cols = np.ceil(counts.max(axis=0) / P).astype(np.int64)      # [N_TILES, N_BANKS]
    NCH = int(cols.sum())                                         # chunks per core

    # group boundaries for gather calls: groups of GROUP node tiles
    groups = [
        list(range(g, min(g + GROUP, N_TILES))) for g in range(0, N_TILES, GROUP)
    ]

    # re-order chunks canonically: for g: for b: for t in g: cols[t][b] chunks
    # chunk_index[t][b] = start in canonical order
    ch_index = np.zeros((N_TILES, N_BANKS), dtype=np.int64)
    call_plan = []  # per group: list of (b, ch0, n_chunks) with ch0 canonical
    group_start = []
    cursor = 0
    for g in groups:
        group_start.append(cursor)
        for b in range(N_BANKS):
            cw = int(cols[np.array(g), b].sum())
            if cw:
                call_plan.append((len(group_start) - 1, b, cursor, cw))
            for t in g:
                ch_index[t, b] = cursor
                cursor += int(cols[t, b])
    assert cursor == NCH

    # per-core edge placement
    in_maps = []
    for c in range(N_CORES):
        lo = np.searchsorted(key_o, c * N_TILES * N_BANKS)
        hi = np.searchsorted(key_o, (c + 1) * N_TILES * N_BANKS)
        ks = key_o[lo:hi] - c * N_TILES * N_BANKS       # (t * N_BANKS + b)
        # position within the (t,b) run for each edge
        run_starts = np.concatenate(
            [[0], np.cumsum(np.bincount(ks, minlength=N_TILES * N_BANKS))]
        )[:-1]
        i_local = np.arange(hi - lo) - run_starts[ks]
        chv = ch_index.ravel()[ks] + i_local // P
        pv = (i_local % P).astype(np.int64)

        erel = np.zeros((P, NCH), dtype=np.float32)
        enrm = np.zeros((P, NCH), dtype=np.float32)
        idx16 = np.zeros((16, NCH * 8), dtype=np.int16)
        drel = (d_o[lo:hi] - (c * NT + (ks // N_BANKS) * P)).astype(np.float32)
        erel[pv, chv] = drel
        enrm[pv, chv] = n_o[lo:hi]
        bank_lo = np.asarray(BANK_BOUNDS, dtype=np.int64)[(ks % N_BANKS)]
        bloc = (s_o[lo:hi] - bank_lo).astype(np.int16)
        idx16[pv % 16, chv * 8 + pv // 16] = bloc
        idx128 = np.tile(idx16, (8, 1))

        xT = np.zeros((F_PAD, NT), dtype=BF16)
        xT[:F_IN, :] = x[c * NT : (c + 1) * NT, :].T.astype(BF16)
        W1p = np.zeros((F_PAD, HID), dtype=BF16)
        W1p[:F_IN, :] = W1.astype(BF16)
        W2p = np.zeros((HID, HID2), dtype=BF16)
        W2p[:, :C] = W2.astype(BF16)
        iota = np.tile(np.arange(P, dtype=BF16), (P, 1))
        own = dinv[c * NT : (c + 1) * NT] ** 2
        dv2 = np.zeros((P, N_TILES), dtype=np.float32)
        for t in range(N_TILES):
            tsz = min(P, NT - t * P)
            dv2[:tsz, t] = own[t * P : t * P + tsz]
        in_maps.append(
            dict(
                xT=np.ascontiguousarray(xT),
                dinv2=dv2,
                ident=np.eye(P, dtype=BF16),
                W1p=W1p,
                b1=b1.reshape(HID, 1).astype(np.float32),
                W2p=W2p,
                b2bc=np.tile(b2.reshape(1, C), (P, 1)).astype(np.float32),
                iota=iota,
                idx16=idx128,
                erel=erel,
                enrm=enrm,
            )
        )
    plan = dict(NCH=NCH, cols=cols, ch_index=ch_index, groups=groups,
                group_start=group_start, call_plan=call_plan)
    return in_maps, plan


def _build(plan):
    NCH = plan["NCH"]
    cols = plan["cols"]
    ch_index = plan["ch_index"]
    groups = plan["groups"]
    group_start = plan["group_start"]
    call_plan = plan["call_plan"]
    f32 = mybir.dt.float32
    bf16 = mybir.dt.bfloat16

    nc = bacc.Bacc("TRN2", target_bir_lowering=False, num_devices=N_CORES)
    xT = nc.declare_dram_parameter("xT", [F_PAD, NT], bf16, isOutput=False)
    W1p = nc.declare_dram_parameter("W1p", [F_PAD, HID], bf16, isOutput=False)
    b1p = nc.declare_dram_parameter("b1", [HID, 1], f32, isOutput=False)
    W2p = nc.declare_dram_parameter("W2p", [HID, HID2], bf16, isOutput=False)
    b2bc = nc.declare_dram_parameter("b2bc", [P, C], f32, isOutput=False)
    iota_in = nc.declare_dram_parameter("iota", [P, P], bf16, isOutput=False)
    ident_in = nc.declare_dram_parameter("ident", [P, P], bf16, isOutput=False)
    dinv2_in = nc.declare_dram_parameter("dinv2", [P, N_TILES], f32, isOutput=False)
    idx16_in = nc.declare_dram_parameter("idx16", [P, NCH * 8], mybir.dt.int16, isOutput=False)
    erel_in = nc.declare_dram_parameter("erel", [P, NCH], f32, isOutput=False)
    enrm_in = nc.declare_dram_parameter("enrm", [P, NCH], f32, isOutput=False)
    out_p = nc.declare_dram_parameter("out", [NT, C], f32, isOutput=True)

    h1_local = nc.dram_tensor("h1_local", [NT, HID2], bf16)
    h1_full = nc.dram_tensor("h1_full", [N, HID2], bf16, addr_space="Shared")
    h2_local = nc.dram_tensor("h2_local", [NT, HID2], bf16)
    h2_full = nc.dram_tensor("h2_full", [N, HID2], bf16, addr_space="Shared")

    rg = [list(range(N_CORES))]
    max_cols_g = max(
        int(cols[np.array(g), :].sum()) for g in groups
    )

    with tile.TileContext(nc) as tc, ExitStack() as ctx:
        consts = ctx.enter_context(tc.tile_pool(name="consts", bufs=1))
        xpool = ctx.enter_context(tc.tile_pool(name="xpool", bufs=3))
        psA = ctx.enter_context(tc.tile_pool(name="psA", bufs=2, space="PSUM"))
        h1out = ctx.enter_context(tc.tile_pool(name="h1out", bufs=3))
        big = ctx.enter_context(tc.tile_pool(name="big", bufs=1))
        gpool = ctx.enter_context(tc.tile_pool(name="gpool", bufs=2))
        spool = ctx.enter_context(tc.tile_pool(name="spool", bufs=3))
        mpool = ctx.enter_context(tc.tile_pool(name="mpool", bufs=3))
        ps1 = ctx.enter_context(tc.tile_pool(name="ps1", bufs=2, space="PSUM"))
        smalls = ctx.enter_context(tc.tile_pool(name="smalls", bufs=4))

        # ---- constants ----
        iota_t = consts.tile([P, P], bf16)
        nc.sync.dma_start(out=iota_t[:], in_=iota_in[:])
        ident_t = consts.tile([P, P], bf16)
        nc.sync.dma_start(out=ident_t[:], in_=ident_in[:])
        dinv2_t = consts.tile([P, N_TILES], f32)
        nc.sync.dma_start(out=dinv2_t[:], in_=dinv2_in[:])
        W1t = consts.tile([P, 5 * HID], bf16)
        W1t3 = W1t[:].rearrange("p (c h) -> p c h", c=5)
        nc.sync.dma_start(
            out=W1t3, in_=W1p[:].rearrange("(c p) h -> p c h", c=5)
        )
        b1t = consts.tile([HID, 1], f32)
        nc.sync.dma_start(out=b1t[:], in_=b1p[:])
        W2t = consts.tile([HID, HID2], bf16)
        nc.sync.dma_start(out=W2t[:], in_=W2p[:])
        b2t = consts.tile([P, C], f32)
        nc.sync.dma_start(out=b2t[:], in_=b2bc[:])
        idxt = consts.tile([P, NCH * 8], mybir.dt.int16)
        nc.sync.dma_start(out=idxt[:], in_=idx16_in[:])
        erelt = consts.tile([P, NCH], f32)
        nc.sync.dma_start(out=erelt[:], in_=erel_in[:])
        enrmt = consts.tile([P, NCH], f32)
        nc.sync.dma_start(out=enrmt[:], in_=enrm_in[:])

        xT3 = xT[:].rearrange("(c p) n -> p c n", c=5)

        # ---- phase 1: h1_local = x @ W1 ----
        for t in range(N_TILES):
            t0 = t * P
            tsz = min(P, NT - t0)
            xt = xpool.tile([P, 5 * P], bf16, tag="xt")
            xt3 = xt[:].rearrange("p (c n) -> p c n", c=5)
            nc.sync.dma_start(out=xt3[:, :, :tsz], in_=xT3[:, :, t0 : t0 + tsz])
            pa = psA.tile([P, HID], f32)
            for cb in range(5):
                nc.tensor.matmul(
                    out=pa[:tsz, :],
                    lhsT=xt3[:, cb, :tsz],
                    rhs=W1t3[:, cb, :],
                    start=(cb == 0),
                    stop=(cb == 4),
                )
            h1sb = h1out.tile([P, HID], bf16, tag="h1sb")
            nc.scalar.activation(
                out=h1sb[:tsz, :], in_=pa[:tsz, :],
                func=mybir.ActivationFunctionType.Copy,
            )
            nc.sync.dma_start(
                out=h1_local[t0 : t0 + tsz, 0:HID], in_=h1sb[:tsz, :]
            )

        if SIM_MODE:
            nc.sync.dma_start(out=h1_full[0:NT, :], in_=h1_local[:])
        else:
            nc.gpsimd.collective_compute(
                "AllGather", mybir.AluOpType.bypass, replica_groups=rg,
                ins=[h1_local[:]], outs=[h1_full[:]],
            )

        # persistent relu(h1_agg)^T  [HID, NT]
        h1rT = big.tile([HID, NT], bf16)

        def conv(layer, h_full, h_local, gi):
            """Aggregation sweep. layer=1: out h1rT (transposed, relu+b1).
            layer=2: +b2 into the batched logits buffer."""
            for gidx, g in enumerate(groups):
                gs = group_start[gidx]
                gout = gpool.tile([P, max_cols_g * HID2], bf16, tag="gout")
                g3 = gout[:].rearrange("p (c h) -> p c h", c=max_cols_g)
                for (gg, b, ch0, cw) in call_plan:
                    if gg != gidx:
                        continue
                    loc = ch0 - gs
                    nc.gpsimd.dma_gather(
                        out_ap=g3[:, loc : loc + cw, :],
                        in_ap=h_full[BANK_BOUNDS[b] : BANK_BOUNDS[b + 1], :],
                        idxs_ap=idxt[:, ch0 * 8 : (ch0 + cw) * 8],
                        num_idxs=cw * P,
                        num_idxs_reg=cw * P,
                        elem_size=HID2,
                        single_packet=False,
                    )
                for t in g:
                    t0 = t * P
                    tsz = min(P, NT - t0)
                    n_ch_t = int(cols[t, :].sum()) + 1
                    if layer == 1:
                        pt = ps1.tile([HID, P], f32, tag="ps_l1")
                    else:
                        pt = ps1.tile([P, HID], f32, tag="ps_l2")
                    # self-loop diagonal term: D = diag(dinv^2) over this tile,
                    # own rows come from the core-local h tensor
                    hown = mpool.tile([P, HID2], bf16, tag="hown")
                    nc.sync.dma_start(
                        out=hown[:tsz, :], in_=h_local[t0 : t0 + tsz, :]
                    )
                    DD = spool.tile([P, P], bf16, tag="DD")
                    nc.vector.tensor_scalar(
                        out=DD[:], in0=ident_t[:], scalar1=dinv2_t[:, t : t + 1],
                        scalar2=None, op0=mybir.AluOpType.mult,
                    )
                    if layer == 1:
                        nc.tensor.matmul(
                            out=pt[:], lhsT=hown[:, 0:HID], rhs=DD[:],
                            start=True, stop=(n_ch_t == 1),
                        )
                    else:
                        nc.tensor.matmul(
                            out=pt[:], lhsT=DD[:], rhs=hown[:, 0:HID],
                            start=True, stop=(n_ch_t == 1),
                        )
                    k = 1
                    for b in range(N_BANKS):
                        cw = int(cols[t, b])
                        if cw == 0:
                            continue
                        ch0 = int(ch_index[t, b])
                        loc = ch0 - gs
                        for j in range(cw):
                            ch = ch0 + j
                            # norm-scaled one-hot: S[p, n] = (iota==dst_rel[p]) * norm[p]
                            SS = spool.tile([P, P], bf16, tag="SS")
                            nc.vector.tensor_scalar(
                                out=SS[:], in0=iota_t[:],
                                scalar1=erelt[:, ch : ch + 1],
                                scalar2=enrmt[:, ch : ch + 1],
                                op0=mybir.AluOpType.is_equal,
                                op1=mybir.AluOpType.mult,
                            )
                            if layer == 1:
                                nc.tensor.matmul(
                                    out=pt[:],
                                    lhsT=g3[:, loc + j, 0:HID],
                                    rhs=SS[:],
                                    start=False,
                                    stop=(k == n_ch_t - 1),
                                )
                            else:
                                nc.tensor.matmul(
                                    out=pt[:],
                                    lhsT=SS[:],
                                    rhs=g3[:, loc + j, 0:HID],
                                    start=False,
                                    stop=(k == n_ch_t - 1),
                                )
                            k += 1
                    if layer == 1:
                        nc.scalar.activation(
                            out=h1rT[:, t0 : t0 + tsz], in_=pt[:, :tsz],
                            func=mybir.ActivationFunctionType.Relu,
                            bias=b1t[:],
                        )
                    else:
                        # L = psum + b2 into the batched logits buffer
                        nc.vector.tensor_tensor(
                            out=Lb3[:tsz, t, :], in0=pt[:tsz, :C],
                            in1=b2t[:tsz, :], op=mybir.AluOpType.add,
                        )

        conv(1, h1_full, h1_local, 0)

        # ---- layer 2 linear: h2_local = relu(h1_agg) @ W2 (zero-padded cols) ----
        for t in range(N_TILES):
            t0 = t * P
            tsz = min(P, NT - t0)
            pb = psA.tile([P, HID2], f32, tag="ps_l2lin")
            nc.tensor.matmul(
                out=pb[:tsz, :], lhsT=h1rT[:, t0 : t0 + tsz], rhs=W2t[:],
                start=True, stop=True,
            )
            h2sb = h1out.tile([P, HID2], bf16, tag="h2sb")
            nc.scalar.activation(
                out=h2sb[:tsz, :], in_=pb[:tsz, :],
                func=mybir.ActivationFunctionType.Copy,
            )
            nc.sync.dma_start(out=h2_local[t0 : t0 + tsz, :], in_=h2sb[:tsz, :])

        if SIM_MODE:
            nc.sync.dma_start(out=h2_full[0:NT, :], in_=h2_local[:])
        else:
            nc.gpsimd.collective_compute(
                "AllGather", mybir.AluOpType.bypass, replica_groups=rg,
                ins=[h2_local[:]], outs=[h2_full[:]],
            )

        Lbig = big.tile([P, N_TILES * C], f32)
        Lb3 = Lbig[:].rearrange("p (t c) -> p t c", t=N_TILES)

        conv(2, h2_full, h2_local, 1)

        # ---- batched log_softmax over all tiles ----
        negm = big.tile([P, N_TILES], f32)
        nc.vector.tensor_reduce(
            out=negm[:], in_=Lb3, axis=mybir.AxisListType.X,
            op=mybir.AluOpType.max, negate=True,
        )
        # Lc = L - max (3D broadcast of negm), in place
        Lc = Lbig
        Lc3 = Lb3
        nc.vector.tensor_tensor(
            out=Lc3, in0=Lb3, in1=negm[:].to_broadcast([P, N_TILES, C]),
            op=mybir.AluOpType.add,
        )
        Eb = big.tile([P, N_TILES * C], f32)
        nc.scalar.activation(
            out=Eb[:], in_=Lc[:], func=mybir.ActivationFunctionType.Exp,
        )
        sums = big.tile([P, N_TILES], f32)
        nc.vector.tensor_reduce(
            out=sums[:], in_=Eb[:].rearrange("p (t c) -> p t c", t=N_TILES),
            axis=mybir.AxisListType.X, op=mybir.AluOpType.add,
        )
        lns = big.tile([P, N_TILES], f32)
        nc.scalar.activation(
            out=lns[:], in_=sums[:], func=mybir.ActivationFunctionType.Ln,
        )
        # out = Lc - ln(sum)
        nc.vector.tensor_tensor(
            out=Lc3, in0=Lc3, in1=lns[:].to_broadcast([P, N_TILES, C]),
            op=mybir.AluOpType.subtract,
        )
        # two DMAs: full tiles then the 84-row tail (rows beyond NT are garbage)
        nc.sync.dma_start(
            out=out_p[0 : (N_TILES - 1) * P, :].rearrange("(t p) c -> p t c", t=N_TILES - 1),
            in_=Lc3[:, : N_TILES - 1, :],
        )
        last0 = (N_TILES - 1) * P
        nc.sync.dma_start(
            out=out_p[last0:NT, :], in_=Lc3[: NT - last0, N_TILES - 1, :],
        )

    nc.compile()
    return nc


def kernel(x, src, dst, W1, b1, W2, b2):
    in_maps, plan = _preprocess(x, src, dst, W1, b1, W2, b2)
    nc = _build(plan)
    res = run_bass_kernel_spmd(
        nc, in_maps, list(range(N_CORES)), trace=PROFILE
    )
    _LAST_RESULTS["exec_time_ns"] = getattr(res, "exec_time_ns", None)
    _LAST_RESULTS["profile_json"] = getattr(res, "profile_json", None)
    out = np.concatenate([res.results[c]["out"] for c in range(N_CORES)], axis=0)
    return out.astype(np.float32)
